# revision 10
# baseline (speedup 1.0000x reference)
"""BMMRemapper Trainium2 kernel (v3).

Math: out[n,c,q] = sum_k x[n,c,k] * mat[n,q,k]; mat has 4 nonzeros per row q
(bilinear corners lin, lin+1, lin+48, lin+49 with weights (1-a)(1-b), (1-a)b,
a(1-b), ab, zeroed by an all-batch disk mask).

Design (per core, batch-parallel; N=8 = n_cores, no cross-core comms):
  - Quad-row table xq staged in fp16 [2304, 512]: row k = the four corner
    rows [x^T[k] | x^T[k+1] | x^T[k+48] | x^T[k+49]] (1 KB per pixel).
  - dma_gather (Q7 custom SWDGE op) fetches ALL pixels of a chunk in ONE
    instruction (idx int16, 16-partition-wrapped layout) — amortizes the
    ~1 us/instruction SWDGE fixed cost that bound v1 (18 indirect calls).
    The wrapped idx layout is computed directly on 128 partitions from a
    host-staged group-replicated copy of the grid (gidx), so no on-chip
    partition shuffle is needed.
  - Combine in fp16 with three wide TT passes per chunk, all eligible for
    DVE 2x_1p mode (2 elem/cycle). Per-(p,t) weights stay fast-mode-
    eligible via duplication: each weight is stored twice adjacently
    (innermost AP dim [1,2], step 1) with channel broadcast on stride-0
    middle dims:
      m = G * Wdup     [p, (t, rv, rh, c2, j)]
      s = m_rv0 + m_rv1
      o = s_rh0 + s_rh1  -> store
  - Mask folded into the weights; output stored fp16, host upcasts.

Layouts (q = output pixel, 0..2303; t = q//128; p = q%128):
  xq     (2304, 512) fp16 : quad-row table.
  gidx   (128, 288)  f32  : grid for the idx chain, [p, 2s+coord] where
                            s = t*8 + (p'//16), pixel p' = (s%8)*16 + p%16
                            (rows replicated across the eight 16-partition
                            groups, as dma_gather's idx layout requires).
  gcoef  (128, 36)   f32  : own-batch grid, [p, 2*t+coord].
  gall   (128, 288)  f32  : all-batch grid, [p, 16*t + 2*m + coord].
  outp   (128, 2304) fp16 : [p, t*128 + c]  (host re-permutes to (c, q)).
"""

import numpy as np

N, H, W, C = 8, 48, 48, 128
HW = H * W            # 2304
NT = HW // 128        # 18
EPS = 1e-5
CLIP_HI = float(np.float32(float(H - 1) - EPS))  # 46.99999 (f32)

# (t0, t1, swdge_queue) chunks. Gathers on queues 1-3 dispatch in ~70 ns
# and run asynchronously on their own Q7 core pairs (3-way parallel
# descriptor generation); queue 0 dispatches synchronously, so its chunks
# go last where the engine hold doesn't block anything.
CHUNKS = [(0, 2, 1), (2, 4, 2), (4, 6, 3), (6, 10, 1), (10, 14, 2),
          (14, 18, 3)]

_CACHE = {}


def _build_nc():
    from contextlib import ExitStack

    import concourse.bacc as bacc
    import concourse.mybir as mybir
    import concourse.tile as tile

    dt = mybir.dt
    f32, f16, i16 = dt.float32, dt.float16, dt.int16
    i32 = dt.int32
    Alu = mybir.AluOpType

    nc = bacc.Bacc(
        "TRN2", target_bir_lowering=False, debug=False, num_devices=N,
        num_swdge_queues=4,
    )

    xq = nc.dram_tensor("xq", [HW, 4 * C], f16, kind="ExternalInput")
    gidx = nc.dram_tensor("gidx", [128, 2 * 8 * NT], f32, kind="ExternalInput")
    gcoef = nc.dram_tensor("gcoef", [128, 2 * NT], f32, kind="ExternalInput")
    gall = nc.dram_tensor("gall", [128, 16 * NT], f32, kind="ExternalInput")
    outp = nc.dram_tensor("outp", [128, HW], f16, kind="ExternalOutput")

    with tile.TileContext(nc) as tc, ExitStack() as ctx:
        pool = ctx.enter_context(tc.tile_pool(name="p", bufs=1))

        # ---- load grid layouts (HWDGE) ----
        g_idx = pool.tile([128, 16 * NT], f32)
        nc.sync.dma_start(g_idx[:], gidx.ap())
        g_coef = pool.tile([128, 2 * NT], f32)
        g_all = pool.tile([128, 16 * NT], f32)
        nc.sync.dma_start(g_coef[:], gcoef.ap())
        nc.sync.dma_start(g_all[:], gall.ap())

        # ---- gather-index chain (critical path; 16-wrapped layout) ----
        # fast floor: int cast of (x - 0.5) rounds-to-nearest on HW, which is
        # exact floor for clipped x (CoreSim truncates -> sim numerics differ,
        # but stay in-bounds; HW is ground truth).
        cab2 = pool.tile([128, 16 * NT], f32)
        nc.vector.tensor_scalar(cab2[:], g_idx[:], EPS, CLIP_HI, Alu.max, Alu.min)
        ti2 = pool.tile([128, 16 * NT], i32)
        nc.vector.tensor_scalar(ti2[:], cab2[:], -0.5, None, Alu.add)
        idx16 = pool.tile([128, 8 * NT], i16)
        nc.vector.scalar_tensor_tensor(
            idx16[:], ti2[:, 0::2], float(W), ti2[:, 1::2], Alu.mult, Alu.add
        )

        # ---- quad gathers: ONE dma_gather per chunk ----
        gts = []
        for ci, (t0, t1, qn) in enumerate(CHUNKS):
            k = t1 - t0
            gt_c = pool.tile([128, k * 512], f16, tag=f"G{ci}")
            nc.gpsimd.dma_gather(
                out_ap=gt_c[:].rearrange("p (t e) -> p t e", e=512),
                in_ap=xq.ap(),
                idxs_ap=idx16[:, 8 * t0 : 8 * t1],
                num_idxs=128 * k,
                num_idxs_reg=128 * k,
                elem_size=512,
                queue_num=qn,
            )
            gts.append(gt_c)

        # ---- coefficient chain ([128, NT] per quantity) ----
        cab = pool.tile([128, 2 * NT], f32)
        nc.vector.tensor_scalar(cab[:], g_coef[:], EPS, CLIP_HI, Alu.max, Alu.min)
        tic = pool.tile([128, 2 * NT], i32)
        nc.vector.tensor_scalar(tic[:], cab[:], -0.5, None, Alu.add)
        tf = pool.tile([128, 2 * NT], f32)
        nc.vector.tensor_copy(tf[:], tic[:])

        # mask: AND over all batches of in-bounds test
        g_all3 = g_all[:].rearrange("p (t m) -> p t m", m=16)
        mn = pool.tile([128, NT], f32)
        mx = pool.tile([128, NT], f32)
        nc.vector.tensor_reduce(mn[:], g_all3, mybir.AxisListType.X, Alu.min)
        nc.vector.tensor_reduce(mx[:], g_all3, mybir.AxisListType.X, Alu.max)
        mge = pool.tile([128, NT], f32)
        mle = pool.tile([128, NT], f32)
        nc.vector.tensor_scalar(mge[:], mn[:], -0.5, None, Alu.is_ge)
        nc.vector.tensor_scalar(mle[:], mx[:], float(H) - 0.5, None, Alu.is_le)
        mask = pool.tile([128, NT], f32)
        nc.vector.tensor_tensor(mask[:], mge[:], mle[:], Alu.mult)

        # weights, mask folded in
        fr = pool.tile([128, 2 * NT], f32)   # fractions (a, b interleaved)
        nc.vector.tensor_tensor(fr[:], cab[:], tf[:], Alu.subtract)
        a = fr[:, 0::2]
        b = fr[:, 1::2]
        fb0 = pool.tile([128, NT], f32)  # 1-b
        nc.vector.tensor_scalar(fb0[:], b, -1.0, 1.0, Alu.mult, Alu.add)
        fa0 = pool.tile([128, NT], f32)  # 1-a
        nc.vector.tensor_scalar(fa0[:], a, -1.0, 1.0, Alu.mult, Alu.add)
        am = pool.tile([128, NT], f32)   # a*mask
        a0m = pool.tile([128, NT], f32)  # (1-a)*mask
        nc.vector.tensor_tensor(am[:], a, mask[:], Alu.mult)
        nc.vector.tensor_tensor(a0m[:], fa0[:], mask[:], Alu.mult)

        w4 = []
        for nm, wa, wb in (("w00", a0m, fb0), ("w01", a0m, None),
                           ("w10", am, fb0), ("w11", am, None)):
            wt = pool.tile([128, NT], f32, tag=nm)
            nc.vector.tensor_tensor(
                wt[:], wa[:], wb[:] if wb is not None else b, Alu.mult
            )
            w4.append(wt)

        # wd[p, 8t + 4rv + 2rh + j] = w_{rv,rh}[p, t] (fp16, duplicated j=0,1
        # so the combine's weight AP has innermost [1,2] -> DVE 2x_1p mode)
        wd = pool.tile([128, 8 * NT], f16)
        for r, wt in enumerate(w4):
            nc.vector.tensor_copy(
                wd[:].rearrange("p (t r j) -> p t r j", r=4, j=2)[:, :, r, :],
                wt[:].rearrange("p (t j) -> p t j", j=1).broadcast_to([128, NT, 2]),
            )

        # ---- combine per chunk: 3 wide fp16 TT passes, all 2x_1p ----
        o_a = pool.tile([128, 6 * 128], f16, tag="o_a")
        o_b = pool.tile([128, 12 * 128], f16, tag="o_b")
        for ci, (t0, t1, _qn) in enumerate(CHUNKS):
            k = t1 - t0
            g5 = gts[ci][:].rearrange(
                "p (t rv rh c2 j) -> p t rv rh c2 j", t=k, rv=2, rh=2, c2=64, j=2
            )
            wd5 = (
                wd[:, 8 * t0 : 8 * t1]
                .rearrange("p (t rv rh j) -> p t rv rh j", rv=2, rh=2, j=2)
                .unsqueeze(4)
                .broadcast_to([128, k, 2, 2, 64, 2])
            )
            m = pool.tile([128, k * 512], f16, tag=f"m{ci}")
            m5 = m[:].rearrange(
                "p (t rv rh c2 j) -> p t rv rh c2 j", t=k, rv=2, rh=2, c2=64, j=2
            )
            nc.vector.tensor_tensor(m5, g5, wd5, Alu.mult)

            m3 = m[:].rearrange("p (t rv x) -> p t rv x", rv=2, x=256)
            s = pool.tile([128, k * 256], f16, tag=f"s{ci}")
            s3 = s[:].rearrange("p (t x) -> p t x", x=256)
            nc.vector.tensor_tensor(s3, m3[:, :, 0, :], m3[:, :, 1, :], Alu.add)

            s2 = s[:].rearrange("p (t rh c) -> p t rh c", rh=2, c=128)
            ob, b0 = (o_a, 0) if t1 <= 6 else (o_b, 6)
            o3 = ob[:, 128 * (t0 - b0) : 128 * (t1 - b0)].rearrange(
                "p (t c) -> p t c", c=128
            )
            nc.vector.tensor_tensor(o3, s2[:, :, 0, :], s2[:, :, 1, :], Alu.add)
            if t1 == 6:
                nc.sync.dma_start(outp.ap()[:, : 128 * 6], o_a[:])
            elif t1 == NT:
                nc.sync.dma_start(outp.ap()[:, 128 * 6 :], o_b[:])

    nc.compile()
    return nc


def _get_nc():
    if "nc" not in _CACHE:
        _CACHE["nc"] = _build_nc()
    return _CACHE["nc"]


def _stage_inputs(x, grid):
    """Build the per-core input maps (pure data movement / fp16 cast)."""
    x = np.ascontiguousarray(x, dtype=np.float32)
    grid = np.ascontiguousarray(grid, dtype=np.float32)
    xr = x.reshape(N, C, HW)
    gr = grid.reshape(N, HW, 2)

    # quad-row table in fp16: xq[n][k] = [xT[k], xT[k+1], xT[k+48], xT[k+49]]
    xt = np.zeros((N, HW + W + 2, C), dtype=np.float32)
    xt[:, :HW] = xr.transpose(0, 2, 1)
    xq = np.empty((N, HW, 4 * C), dtype=np.float16)
    xq[:, :, 0 * C : 1 * C] = xt[:, 0:HW]
    xq[:, :, 1 * C : 2 * C] = xt[:, 1 : HW + 1]
    xq[:, :, 2 * C : 3 * C] = xt[:, W : HW + W]
    xq[:, :, 3 * C : 4 * C] = xt[:, W + 1 : HW + W + 1]

    # gidx[p, 2s+c] = gr[q(s, p%16), c], q(s, r) = (s//8)*128 + (s%8)*16 + r
    s_ = np.arange(8 * NT)
    r_ = np.arange(16)
    qm = (s_[None, :] // 8) * 128 + (s_[None, :] % 8) * 16 + r_[:, None]  # [16,144]
    gidx16 = gr[:, qm, :].reshape(N, 16, 2 * 8 * NT)          # [n, 16, 288]
    gidx = np.ascontiguousarray(np.tile(gidx16, (1, 8, 1)))   # [n, 128, 288]

    # gcoef[n][p, 2t+c] = gr[n, t*128+p, c]
    gc = gr.reshape(N, NT, 128, 2).transpose(0, 2, 1, 3)  # [n, p, t, c]
    gcoef = np.ascontiguousarray(gc.reshape(N, 128, 2 * NT))

    # gall[p, 16t+2m+c] = gr[m, t*128+p, c]   (same for all cores)
    ga = gr.reshape(N, NT, 128, 2).transpose(2, 1, 0, 3)  # [p, t, m, c]
    gall = np.ascontiguousarray(ga.reshape(128, 16 * NT))

    return [
        {"xq": xq[n], "gidx": gidx[n], "gcoef": gcoef[n], "gall": gall}
        for n in range(N)
    ]


def _unstage_output(results):
    """results[n]["outp"] is (128, 2304) fp16 = [p, t*128+c] -> (N, C, H, W)."""
    out = np.empty((N, C, H, W), dtype=np.float32)
    for n in range(N):
        o = results[n]["outp"].astype(np.float32).reshape(128, NT, C)
        out[n] = o.transpose(2, 1, 0).reshape(C, H, W)   # [c, q=t*128+p]
    return out


def kernel(x, grid):
    from concourse import bass_utils

    nc = _get_nc()
    in_maps = _stage_inputs(x, grid)
    res = bass_utils.run_bass_kernel_spmd(nc, in_maps, core_ids=list(range(N)))
    return _unstage_output(res.results)


# revision 11
# speedup vs baseline: 1.0022x; 1.0022x over previous
"""BMMRemapper Trainium2 kernel (v3).

Math: out[n,c,q] = sum_k x[n,c,k] * mat[n,q,k]; mat has 4 nonzeros per row q
(bilinear corners lin, lin+1, lin+48, lin+49 with weights (1-a)(1-b), (1-a)b,
a(1-b), ab, zeroed by an all-batch disk mask).

Design (per core, batch-parallel; N=8 = n_cores, no cross-core comms):
  - Quad-row table xq staged in fp16 [2304, 512]: row k = the four corner
    rows [x^T[k] | x^T[k+1] | x^T[k+48] | x^T[k+49]] (1 KB per pixel).
  - dma_gather (Q7 custom SWDGE op) fetches ALL pixels of a chunk in ONE
    instruction (idx int16, 16-partition-wrapped layout) — amortizes the
    ~1 us/instruction SWDGE fixed cost that bound v1 (18 indirect calls).
    The wrapped idx layout is computed directly on 128 partitions from a
    host-staged group-replicated copy of the grid (gidx), so no on-chip
    partition shuffle is needed.
  - Combine in fp16 with three wide TT passes per chunk, all eligible for
    DVE 2x_1p mode (2 elem/cycle). Per-(p,t) weights stay fast-mode-
    eligible via duplication: each weight is stored twice adjacently
    (innermost AP dim [1,2], step 1) with channel broadcast on stride-0
    middle dims:
      m = G * Wdup     [p, (t, rv, rh, c2, j)]
      s = m_rv0 + m_rv1
      o = s_rh0 + s_rh1  -> store
  - Mask folded into the weights; output stored fp16, host upcasts.

Layouts (q = output pixel, 0..2303; t = q//128; p = q%128):
  xq     (2304, 512) fp16 : quad-row table.
  gidx   (128, 288)  f32  : grid for the idx chain, [p, 2s+coord] where
                            s = t*8 + (p'//16), pixel p' = (s%8)*16 + p%16
                            (rows replicated across the eight 16-partition
                            groups, as dma_gather's idx layout requires).
  gcoef  (128, 36)   f32  : own-batch grid, [p, 2*t+coord].
  gall   (128, 288)  f32  : all-batch grid, [p, 16*t + 2*m + coord].
  outp   (128, 2304) fp16 : [p, t*128 + c]  (host re-permutes to (c, q)).
"""

import numpy as np

N, H, W, C = 8, 48, 48, 128
HW = H * W            # 2304
NT = HW // 128        # 18
EPS = 1e-5
CLIP_HI = float(np.float32(float(H - 1) - EPS))  # 46.99999 (f32)

# (t0, t1, swdge_queue) chunks. Gathers on queues 1-3 dispatch in ~70 ns
# and run asynchronously on their own Q7 core pairs (3-way parallel
# descriptor generation); queue 0 dispatches synchronously, so its chunks
# go last where the engine hold doesn't block anything.
CHUNKS = [(0, 2, 1), (2, 4, 2), (4, 6, 3), (6, 10, 1), (10, 14, 2),
          (14, 18, 3)]

_CACHE = {}


def _build_nc():
    from contextlib import ExitStack

    import concourse.bacc as bacc
    import concourse.mybir as mybir
    import concourse.tile as tile

    dt = mybir.dt
    f32, f16, i16 = dt.float32, dt.float16, dt.int16
    i32 = dt.int32
    Alu = mybir.AluOpType

    nc = bacc.Bacc(
        "TRN2", target_bir_lowering=False, debug=False, num_devices=N,
        num_swdge_queues=4, dynamic_dma_scratch_size=49152,
    )

    xq = nc.dram_tensor("xq", [HW, 4 * C], f16, kind="ExternalInput")
    gidx = nc.dram_tensor("gidx", [128, 2 * 8 * NT], f32, kind="ExternalInput")
    gcoef = nc.dram_tensor("gcoef", [128, 2 * NT], f32, kind="ExternalInput")
    gall = nc.dram_tensor("gall", [128, 16 * NT], f32, kind="ExternalInput")
    outp = nc.dram_tensor("outp", [128, HW], f16, kind="ExternalOutput")

    with tile.TileContext(nc) as tc, ExitStack() as ctx:
        pool = ctx.enter_context(tc.tile_pool(name="p", bufs=1))

        # ---- load grid layouts (HWDGE) ----
        g_idx = pool.tile([128, 16 * NT], f32)
        nc.sync.dma_start(g_idx[:], gidx.ap())
        g_coef = pool.tile([128, 2 * NT], f32)
        g_all = pool.tile([128, 16 * NT], f32)
        nc.sync.dma_start(g_coef[:], gcoef.ap())
        nc.sync.dma_start(g_all[:], gall.ap())

        # ---- gather-index chain (critical path; 16-wrapped layout) ----
        # fast floor: int cast of (x - 0.5) rounds-to-nearest on HW, which is
        # exact floor for clipped x (CoreSim truncates -> sim numerics differ,
        # but stay in-bounds; HW is ground truth).
        cab2 = pool.tile([128, 16 * NT], f32)
        nc.vector.tensor_scalar(cab2[:], g_idx[:], EPS, CLIP_HI, Alu.max, Alu.min)
        ti2 = pool.tile([128, 16 * NT], i32)
        nc.vector.tensor_scalar(ti2[:], cab2[:], -0.5, None, Alu.add)
        idx16 = pool.tile([128, 8 * NT], i16)
        nc.vector.scalar_tensor_tensor(
            idx16[:], ti2[:, 0::2], float(W), ti2[:, 1::2], Alu.mult, Alu.add
        )

        # ---- quad gathers: ONE dma_gather per chunk ----
        gts = []
        for ci, (t0, t1, qn) in enumerate(CHUNKS):
            k = t1 - t0
            gt_c = pool.tile([128, k * 512], f16, tag=f"G{ci}")
            nc.gpsimd.dma_gather(
                out_ap=gt_c[:].rearrange("p (t e) -> p t e", e=512),
                in_ap=xq.ap(),
                idxs_ap=idx16[:, 8 * t0 : 8 * t1],
                num_idxs=128 * k,
                num_idxs_reg=128 * k,
                elem_size=512,
                queue_num=qn,
            )
            gts.append(gt_c)

        # ---- coefficient chain ([128, NT] per quantity) ----
        cab = pool.tile([128, 2 * NT], f32)
        nc.vector.tensor_scalar(cab[:], g_coef[:], EPS, CLIP_HI, Alu.max, Alu.min)
        tic = pool.tile([128, 2 * NT], i32)
        nc.vector.tensor_scalar(tic[:], cab[:], -0.5, None, Alu.add)
        tf = pool.tile([128, 2 * NT], f32)
        nc.vector.tensor_copy(tf[:], tic[:])

        # mask: AND over all batches of in-bounds test
        g_all3 = g_all[:].rearrange("p (t m) -> p t m", m=16)
        mn = pool.tile([128, NT], f32)
        mx = pool.tile([128, NT], f32)
        nc.vector.tensor_reduce(mn[:], g_all3, mybir.AxisListType.X, Alu.min)
        nc.vector.tensor_reduce(mx[:], g_all3, mybir.AxisListType.X, Alu.max)
        mge = pool.tile([128, NT], f32)
        mle = pool.tile([128, NT], f32)
        nc.vector.tensor_scalar(mge[:], mn[:], -0.5, None, Alu.is_ge)
        nc.vector.tensor_scalar(mle[:], mx[:], float(H) - 0.5, None, Alu.is_le)
        mask = pool.tile([128, NT], f32)
        nc.vector.tensor_tensor(mask[:], mge[:], mle[:], Alu.mult)

        # weights, mask folded in
        fr = pool.tile([128, 2 * NT], f32)   # fractions (a, b interleaved)
        nc.vector.tensor_tensor(fr[:], cab[:], tf[:], Alu.subtract)
        a = fr[:, 0::2]
        b = fr[:, 1::2]
        fb0 = pool.tile([128, NT], f32)  # 1-b
        nc.vector.tensor_scalar(fb0[:], b, -1.0, 1.0, Alu.mult, Alu.add)
        fa0 = pool.tile([128, NT], f32)  # 1-a
        nc.vector.tensor_scalar(fa0[:], a, -1.0, 1.0, Alu.mult, Alu.add)
        am = pool.tile([128, NT], f32)   # a*mask
        a0m = pool.tile([128, NT], f32)  # (1-a)*mask
        nc.vector.tensor_tensor(am[:], a, mask[:], Alu.mult)
        nc.vector.tensor_tensor(a0m[:], fa0[:], mask[:], Alu.mult)

        w4 = []
        for nm, wa, wb in (("w00", a0m, fb0), ("w01", a0m, None),
                           ("w10", am, fb0), ("w11", am, None)):
            wt = pool.tile([128, NT], f32, tag=nm)
            nc.vector.tensor_tensor(
                wt[:], wa[:], wb[:] if wb is not None else b, Alu.mult
            )
            w4.append(wt)

        # wd[p, 8t + 4rv + 2rh + j] = w_{rv,rh}[p, t] (fp16, duplicated j=0,1
        # so the combine's weight AP has innermost [1,2] -> DVE 2x_1p mode)
        wd = pool.tile([128, 8 * NT], f16)
        for r, wt in enumerate(w4):
            nc.vector.tensor_copy(
                wd[:].rearrange("p (t r j) -> p t r j", r=4, j=2)[:, :, r, :],
                wt[:].rearrange("p (t j) -> p t j", j=1).broadcast_to([128, NT, 2]),
            )

        # ---- combine per chunk: 3 wide fp16 TT passes, all 2x_1p ----
        o_a = pool.tile([128, 6 * 128], f16, tag="o_a")
        o_b = pool.tile([128, 12 * 128], f16, tag="o_b")
        for ci, (t0, t1, _qn) in enumerate(CHUNKS):
            k = t1 - t0
            g5 = gts[ci][:].rearrange(
                "p (t rv rh c2 j) -> p t rv rh c2 j", t=k, rv=2, rh=2, c2=64, j=2
            )
            wd5 = (
                wd[:, 8 * t0 : 8 * t1]
                .rearrange("p (t rv rh j) -> p t rv rh j", rv=2, rh=2, j=2)
                .unsqueeze(4)
                .broadcast_to([128, k, 2, 2, 64, 2])
            )
            m = pool.tile([128, k * 512], f16, tag=f"m{ci}")
            m5 = m[:].rearrange(
                "p (t rv rh c2 j) -> p t rv rh c2 j", t=k, rv=2, rh=2, c2=64, j=2
            )
            nc.vector.tensor_tensor(m5, g5, wd5, Alu.mult)

            m3 = m[:].rearrange("p (t rv x) -> p t rv x", rv=2, x=256)
            s = pool.tile([128, k * 256], f16, tag=f"s{ci}")
            s3 = s[:].rearrange("p (t x) -> p t x", x=256)
            nc.vector.tensor_tensor(s3, m3[:, :, 0, :], m3[:, :, 1, :], Alu.add)

            s2 = s[:].rearrange("p (t rh c) -> p t rh c", rh=2, c=128)
            ob, b0 = (o_a, 0) if t1 <= 6 else (o_b, 6)
            o3 = ob[:, 128 * (t0 - b0) : 128 * (t1 - b0)].rearrange(
                "p (t c) -> p t c", c=128
            )
            nc.vector.tensor_tensor(o3, s2[:, :, 0, :], s2[:, :, 1, :], Alu.add)
            if t1 == 6:
                nc.sync.dma_start(outp.ap()[:, : 128 * 6], o_a[:])
            elif t1 == NT:
                nc.sync.dma_start(outp.ap()[:, 128 * 6 :], o_b[:])

    nc.compile()
    return nc


def _get_nc():
    if "nc" not in _CACHE:
        _CACHE["nc"] = _build_nc()
    return _CACHE["nc"]


def _stage_inputs(x, grid):
    """Build the per-core input maps (pure data movement / fp16 cast)."""
    x = np.ascontiguousarray(x, dtype=np.float32)
    grid = np.ascontiguousarray(grid, dtype=np.float32)
    xr = x.reshape(N, C, HW)
    gr = grid.reshape(N, HW, 2)

    # quad-row table in fp16: xq[n][k] = [xT[k], xT[k+1], xT[k+48], xT[k+49]]
    xt = np.zeros((N, HW + W + 2, C), dtype=np.float32)
    xt[:, :HW] = xr.transpose(0, 2, 1)
    xq = np.empty((N, HW, 4 * C), dtype=np.float16)
    xq[:, :, 0 * C : 1 * C] = xt[:, 0:HW]
    xq[:, :, 1 * C : 2 * C] = xt[:, 1 : HW + 1]
    xq[:, :, 2 * C : 3 * C] = xt[:, W : HW + W]
    xq[:, :, 3 * C : 4 * C] = xt[:, W + 1 : HW + W + 1]

    # gidx[p, 2s+c] = gr[q(s, p%16), c], q(s, r) = (s//8)*128 + (s%8)*16 + r
    s_ = np.arange(8 * NT)
    r_ = np.arange(16)
    qm = (s_[None, :] // 8) * 128 + (s_[None, :] % 8) * 16 + r_[:, None]  # [16,144]
    gidx16 = gr[:, qm, :].reshape(N, 16, 2 * 8 * NT)          # [n, 16, 288]
    gidx = np.ascontiguousarray(np.tile(gidx16, (1, 8, 1)))   # [n, 128, 288]

    # gcoef[n][p, 2t+c] = gr[n, t*128+p, c]
    gc = gr.reshape(N, NT, 128, 2).transpose(0, 2, 1, 3)  # [n, p, t, c]
    gcoef = np.ascontiguousarray(gc.reshape(N, 128, 2 * NT))

    # gall[p, 16t+2m+c] = gr[m, t*128+p, c]   (same for all cores)
    ga = gr.reshape(N, NT, 128, 2).transpose(2, 1, 0, 3)  # [p, t, m, c]
    gall = np.ascontiguousarray(ga.reshape(128, 16 * NT))

    return [
        {"xq": xq[n], "gidx": gidx[n], "gcoef": gcoef[n], "gall": gall}
        for n in range(N)
    ]


def _unstage_output(results):
    """results[n]["outp"] is (128, 2304) fp16 = [p, t*128+c] -> (N, C, H, W)."""
    out = np.empty((N, C, H, W), dtype=np.float32)
    for n in range(N):
        o = results[n]["outp"].astype(np.float32).reshape(128, NT, C)
        out[n] = o.transpose(2, 1, 0).reshape(C, H, W)   # [c, q=t*128+p]
    return out


def kernel(x, grid):
    from concourse import bass_utils

    nc = _get_nc()
    in_maps = _stage_inputs(x, grid)
    res = bass_utils.run_bass_kernel_spmd(nc, in_maps, core_ids=list(range(N)))
    return _unstage_output(res.results)
